# revision 18
# baseline (speedup 1.0000x reference)
"""Cross multi-head attention on 8 Trainium2 NeuronCores.

Sharding: batch x head-group. Core c handles batch b = c//4 and heads
4*(c%4) .. 4*(c%4)+3. Wq is tensor-sharded by head (columns), Wo by its
input (head) dim (rows); the 4 partial outputs per batch are summed on
the host.

v4 design notes (205us v3 -> this):
- Phase A packs 2 heads per stationary (no DKP column padding on Wq):
  QT pair tiles are [128 = 2 heads x 64, NQ].  The scores matmul still
  contracts over 128 partitions: rows of the OTHER head in the pair act
  as junk rows, and the kt stationary carries eps rows on the opposite
  half (memset on-chip, not DMAed), so the junk contributes ~1e-4 to
  scaled scores while the PE array reads as fully busy (clock-gate).
- vext carries 65 columns per kv tile (64 centered-V + 1 ones for the
  softmax denominator) instead of 128 eps-padded ones: PV output is
  [65, q] and the denominator sits in PSUM row 64.
- Phase B runs the PV matmuls TWO kt tiles behind the scores matmuls
  and the Schraudolph/DVE exp tiles sit EARLY in the block while the
  last two ACT exp tiles are split in half, so the next block's first
  scores matmul never waits on a PSUM bank held by a late exp.
- exp split: ACT exact Exp 10 tiles, DVE int16-Schraudolph 6 tiles
  (measured end-to-end rel err 1.7e-2 < 2e-2).
- Norm per block (deferred into the next block): ACT Ln[1,QB] ->
  ACT Exp(-ln)[1,QB] -> GPS raq = ra*qmask [1,QB] -> GPS
  partition_broadcast [64,QB] -> DVE scalar_tensor_tensor
  dst = attn(PSUM) * raq64(SBUF).
- DMA: batched instructions (one per logical tensor where possible),
  kt ships only the 64 real rows, output is fp16.  Host sums the 4
  partials per batch in f32 and adds the centered-V mean term
  meanV @ Wo.
"""

import numpy as np
import ml_dtypes

import concourse.bass as bass
import concourse.mybir as mybir
import concourse.tile as tile
from concourse.bass_utils import run_bass_kernel_spmd

F32 = mybir.dt.float32
F16 = mybir.dt.float16
BF16 = mybir.dt.bfloat16
I16 = mybir.dt.int16
AF = mybir.ActivationFunctionType
ALU = mybir.AluOpType
BF = ml_dtypes.bfloat16

B, H, NQ, NKV, D, DK = 2, 16, 2048, 2048, 1024, 64
EPS = 1e-4           # junk-row constant in kt: junk q rows (the pair head's
                     # real data, ~N(0,1)) hit these eps rows -> ~1e-5 noise
HPC = 4              # heads per core
CPB = 4              # cores per batch
KT_TILES = NKV // 128
QT_TILES = NQ // 128
MC = D // 128        # model-dim chunks
QB = 1024            # q block width for scores/exp/attn
NQB = NQ // QB
SCALE = 0.125        # 1/sqrt(DK)
VW = 65              # vext columns per kv tile (64 centered V + 1 ones)

# Schraudolph fast-exp (int16 / bf16 bitpattern): exp(x) ~ bitcast_bf16(
#   int16(A16*x + B16)).  A16 = 2^7/ln2; B16 tuned for min max-rel-err.
A16 = 184.66500888183135
B16 = 127.0 * 128.0 - 4.5
# kt tiles handled by Schraudolph fast-exp on the DVE; early positions so
# the block tail is all-ACT (GpSimd cannot read PSUM -> no exp there)
DVE_EXP_KT = frozenset((1, 3, 5, 7, 9, 11))
SPLIT_EXP_KT = frozenset((14, 15))  # ACT tiles split in half for early
                                    # PSUM release at the block boundary


def _split_excess_waits(nc, limit=1):
    """This walrus build rejects instructions carrying several sem waits.
    Move excess waits onto standalone EventSemaphore instructions placed
    directly before the offender on the same (FIFO) engine queue."""
    n = 0
    for f in nc.m.functions:
        for bb in f.blocks:
            out = []
            for inst in bb.instructions:
                si = inst.sync_info
                waits = list(si.on_wait) if si is not None else []
                if len(waits) > limit:
                    excess, keep = waits[:-limit], waits[-limit:]
                    for w in excess:
                        n += 1
                        out.append(mybir.InstEventSemaphore(
                            name=f"wsplit-{n}-{inst.name}",
                            engine=inst.engine,
                            ins=[], outs=[],
                            sync_info=mybir.SyncInfo(on_wait=[w], on_update=[]),
                        ))
                    si.on_wait = keep
                out.append(inst)
            bb.instructions = out
    return n


def _build_program():
    nc = bass.Bass("TRN2", target_bir_lowering=False, debug=False, num_devices=8)

    d_xt = nc.declare_dram_parameter("xt", [D, NQ], BF16, isOutput=False)
    d_wq = nc.declare_dram_parameter("wq", [128, MC * 256], BF16, isOutput=False)
    d_kt = nc.declare_dram_parameter("kt", [HPC, DK, NKV], BF16, isOutput=False)
    d_vx = nc.declare_dram_parameter("vext", [HPC, 128, KT_TILES * VW], BF16, isOutput=False)
    d_wo = nc.declare_dram_parameter("wo", [128, 2 * D], BF16, isOutput=False)
    d_ng = nc.declare_dram_parameter("negm", [128, KT_TILES], F32, isOutput=False)
    d_ngs = nc.declare_dram_parameter("negs", [128, KT_TILES], F32, isOutput=False)
    d_qm = nc.declare_dram_parameter("qmr", [1, NQ], BF16, isOutput=False)
    d_out = nc.declare_dram_parameter("out", [NQ, D], F16, isOutput=True)

    with tile.TileContext(nc) as tc:
        with (
            tc.tile_pool(name="persist", bufs=1) as pp,
            tc.tile_pool(name="fin", bufs=1) as fp,
            tc.tile_pool(name="sb_small", bufs=2) as sp,
        ):
            # ---- persistent tiles ----
            t_kt = [pp.tile([128, NKV], BF16, name=f"kt{h}", tag=f"kt{h}")
                    for h in range(HPC)]
            # memset the junk halves of kt on-chip (saves 1MB of DMA);
            # even heads of a pair use rows 0..63, odd heads rows 64..127
            for h in range(HPC):
                if h % 2 == 0:
                    nc.gpsimd.memset(t_kt[h][DK:128, :], EPS)
                else:
                    nc.gpsimd.memset(t_kt[h][0:DK, :], EPS)

            # ---- input loads (ordered so Phase A can start immediately) ----
            t_ng = pp.tile([128, KT_TILES], F32, name="negm", tag="negm")
            nc.sync.dma_start(out=t_ng[:, :], in_=d_ng[:, :])
            t_ngs = pp.tile([128, KT_TILES], F32, name="negs", tag="negs")
            nc.sync.dma_start(out=t_ngs[:, :], in_=d_ngs[:, :])
            t_qm = pp.tile([1, NQ], BF16, name="qmr", tag="qmr")
            nc.sync.dma_start(out=t_qm[:, :], in_=d_qm[:, :])
            t_xt = [pp.tile([128, NQ], BF16, name=f"xt{mc}", tag=f"xt{mc}")
                    for mc in range(MC)]
            t_wq = pp.tile([128, MC * 256], BF16, name="wq", tag="wq")
            nc.sync.dma_start(out=t_xt[0][:, :], in_=d_xt[0:128, :])
            nc.sync.dma_start(out=t_wq[:, :], in_=d_wq[:, :])
            for mc in range(1, MC):
                nc.sync.dma_start(out=t_xt[mc][:, :],
                                  in_=d_xt[mc * 128:(mc + 1) * 128, :])
            t_vx = [pp.tile([128, KT_TILES * VW], BF16, name=f"vx{h}", tag=f"vx{h}")
                    for h in range(HPC)]
            for h in range(HPC):
                if h % 2 == 0:
                    nc.sync.dma_start(out=t_kt[h][0:DK, :], in_=d_kt[h, :, :])
                else:
                    nc.sync.dma_start(out=t_kt[h][DK:128, :], in_=d_kt[h, :, :])
                nc.sync.dma_start(out=t_vx[h][:, :], in_=d_vx[h, :, :])
            t_wo = pp.tile([128, 2 * D], BF16, name="wo", tag="wo")
            nc.sync.dma_start(out=t_wo[:, :], in_=d_wo[:, :])

            t_qt = [pp.tile([128, NQ], BF16, name=f"qt{p}", tag=f"qt{p}")
                    for p in range(2)]
            t_on = pp.tile([1, DK], BF16, name="ones", tag="ones")
            nc.vector.memset(t_on[:, :], 1.0)
            t_fAB = fp.tile([128, NQ], BF16, name="attnAB", tag="attnAB")
            t_fCD = fp.tile([128, NQ], BF16, name="attnCD", tag="attnCD")

            # ---- Phase A: QT_pair = Wq_pair^T @ x^T (mc-outer, DMA-paced) ----
            with tc.tile_pool(name="ps_q", bufs=1, space="PSUM") as pqp:
                ps_q = [pqp.tile([128, NQ], F32, name=f"ps_q{p}", tag=f"ps_q{p}")
                        for p in range(2)]
                for mc in range(MC):
                    for pr in range(2):
                        for nb in range(NQ // 512):
                            nc.tensor.matmul(
                                ps_q[pr][:, nb * 512:(nb + 1) * 512],
                                t_wq[:, mc * 256 + pr * 128:mc * 256 + (pr + 1) * 128],
                                t_xt[mc][:, nb * 512:(nb + 1) * 512],
                                start=(mc == 0), stop=(mc == MC - 1),
                            )
                # PSUM -> SBUF bf16, split across engines (A->B critical
                # path; GpSimd cannot read PSUM)
                nc.vector.tensor_copy(t_qt[0][:, 0:NQ // 2], ps_q[0][:, 0:NQ // 2])
                nc.scalar.copy(t_qt[0][:, NQ // 2:], ps_q[0][:, NQ // 2:])
                nc.vector.tensor_copy(t_qt[1][:, 0:NQ // 2], ps_q[1][:, 0:NQ // 2])
                nc.scalar.copy(t_qt[1][:, NQ // 2:], ps_q[1][:, NQ // 2:])

            # ---- Phase B: per (q-block, head) attention ----
            # PV matmuls run TWO kt tiles behind scores; normalization of
            # block i is deferred into block i+1 so no engine queue stalls.
            with (
                tc.tile_pool(name="probs", bufs=2) as prp,
                tc.tile_pool(name="ps_sc", bufs=2, space="PSUM") as scp,
                tc.tile_pool(name="ps_at", bufs=2, space="PSUM") as atp,
            ):
                pend = []  # deferred norm steps of the previous block
                for qh in range(NQB):
                    q0 = qh * QB
                    for h in range(HPC):
                        dst = t_fAB if h < 2 else t_fCD
                        rbase = (h % 2) * DK
                        pr = h // 2
                        probsT = prp.tile([128, KT_TILES * QB], BF16,
                                          name="probsT", tag="probsT")
                        # rows 0..63 attn accum, row 64 denominator, rows
                        # 64..127 later overwritten with the ra broadcast
                        at_ps = atp.tile([128, QB], F32, name="at_ps", tag="at_ps")

                        def at_mms(kt, at_ps=at_ps, probsT=probsT, h=h):
                            for nb in range(QB // 512):
                                nc.tensor.matmul(
                                    at_ps[0:VW, nb * 512:(nb + 1) * 512],
                                    t_vx[h][:, kt * VW:(kt + 1) * VW],
                                    probsT[:, kt * QB + nb * 512:kt * QB + (nb + 1) * 512],
                                    start=(kt == 0), stop=(kt == KT_TILES - 1),
                                )

                        for kt in range(KT_TILES):
                            sc = scp.tile([128, QB], F32, name="sc", tag="sc")
                            split = kt in SPLIT_EXP_KT
                            for nb in range(QB // 512):
                                nc.tensor.matmul(
                                    sc[:, nb * 512:(nb + 1) * 512],
                                    t_kt[h][:, kt * 128:(kt + 1) * 128],
                                    t_qt[pr][:, q0 + nb * 512:q0 + (nb + 1) * 512],
                                    start=True, stop=True,
                                )
                                if split:
                                    nc.scalar.activation(
                                        probsT[:, kt * QB + nb * 512:kt * QB + (nb + 1) * 512],
                                        sc[:, nb * 512:(nb + 1) * 512], AF.Exp,
                                        bias=t_ng[:, kt:kt + 1], scale=SCALE)
                            if not split:
                                pslice = probsT[:, kt * QB:(kt + 1) * QB]
                                if kt in DVE_EXP_KT:
                                    nc.vector.tensor_scalar(
                                        pslice.bitcast(I16), sc[:, :],
                                        A16 * SCALE, t_ngs[:, kt:kt + 1],
                                        ALU.mult, ALU.add,
                                    )
                                else:
                                    nc.scalar.activation(pslice, sc[:, :], AF.Exp,
                                                         bias=t_ng[:, kt:kt + 1],
                                                         scale=SCALE)
                            # drain one deferred norm step of the previous
                            # block every couple of kt tiles
                            if kt in (1, 3, 5, 7, 9, 11) and pend:
                                pend.pop(0)()
                            if kt >= 2:
                                at_mms(kt - 2)
                        at_mms(KT_TILES - 2)
                        at_mms(KT_TILES - 1)

                        # deferred normalization steps (run inside next block):
                        #   ln    = Ln(denom row)        [1,QB] f32   (ACT)
                        #   ra    = Exp(-ln)             [1,QB] bf16  (ACT)
                        #   raq   = ra * qmask row       [1,QB] bf16  (GPS)
                        #   bcast raq -> at_ps[64:128]   (PE ones matmul)
                        #   tmp   = copy attn PSUM->SBUF [64,QB] bf16 (DVE)
                        #   dst   = tmp * at_ps[64:128]  (DVE, one PSUM operand)
                        def mk_norm(at_ps=at_ps, dst=dst, rbase=rbase, q0=q0):
                            st = {}

                            def s_ln():
                                t_ln = sp.tile([1, QB], F32, name="ln", tag="ln")
                                nc.scalar.activation(t_ln[:, :],
                                                     at_ps[DK:DK + 1, :], AF.Ln)
                                st["ln"] = t_ln

                            def s_ra():
                                t_ra = sp.tile([1, QB], BF16, name="ra", tag="ra")
                                nc.scalar.activation(t_ra[:, :], st["ln"][:, :],
                                                     AF.Exp, scale=-1.0)
                                st["ra"] = t_ra

                            def s_raq():
                                t_rq = sp.tile([1, QB], BF16, name="raq", tag="raq")
                                nc.gpsimd.tensor_mul(t_rq[:, :], st["ra"][:, :],
                                                     t_qm[:, q0:q0 + QB])
                                st["raq"] = t_rq

                            def s_bc():
                                for nb in range(QB // 512):
                                    s = slice(nb * 512, (nb + 1) * 512)
                                    nc.tensor.matmul(at_ps[DK:2 * DK, s],
                                                     t_on[:, :],
                                                     st["raq"][:, s],
                                                     start=True, stop=True)

                            def s_tmp():
                                t_tmp = sp.tile([DK, QB], BF16, name="tmp",
                                                tag="tmp")
                                nc.vector.tensor_copy(t_tmp[:, :],
                                                      at_ps[0:DK, :])
                                st["tmp"] = t_tmp

                            def s_fin():
                                nc.vector.tensor_mul(
                                    dst[rbase:rbase + DK, q0:q0 + QB],
                                    st["tmp"][:, :], at_ps[DK:2 * DK, :])

                            return [s_ln, s_ra, s_raq, s_bc, s_tmp, s_fin]

                        pend = mk_norm()
                # drain the deferred normalization of the last block
                for s in pend:
                    s()

            # ---- Phase C: out = attnT_final^T @ Wo_slice ----
            with (
                tc.tile_pool(name="ps_out", bufs=2, space="PSUM") as pop,
                tc.tile_pool(name="outsb", bufs=3) as op,
            ):
                for qt_i in range(QT_TILES):
                    qs = slice(qt_i * 128, (qt_i + 1) * 128)
                    po = pop.tile([128, D], F32, name="po", tag="po")
                    for nb in range(D // 512):
                        s = slice(nb * 512, (nb + 1) * 512)
                        nc.tensor.matmul(po[:, s], t_fAB[:, qs], t_wo[:, s],
                                         start=True, stop=False)
                        nc.tensor.matmul(po[:, s], t_fCD[:, qs], t_wo[:, D + s.start:D + s.stop],
                                         start=False, stop=True)
                    t_out = op.tile([128, D], F16, name="t_out", tag="t_out")
                    if qt_i % 2 == 0 or qt_i >= QT_TILES - 2:
                        nc.vector.tensor_copy(t_out[:, :], po[:, :])
                    else:
                        nc.scalar.copy(t_out[:, :], po[:, :])
                    nc.sync.dma_start(out=d_out[qs, :], in_=t_out[:, :])

    _split_excess_waits(nc, limit=1)
    return nc


_PROGRAM = None


def _get_program():
    global _PROGRAM
    if _PROGRAM is None:
        _PROGRAM = _build_program()
    return _PROGRAM


def _core_inputs(c, x, K, V, Wq, Wo, kv_pad_mask, q_pad_mask):
    b = c // CPB
    g = c % CPB
    hs = slice(HPC * g, HPC * g + HPC)
    xt = np.ascontiguousarray(x[b].T).astype(BF)
    # Wq columns for this group's 4 heads, merged mc-major: [128, MC*256]
    wqs = Wq[:, HPC * DK * g:HPC * DK * (g + 1)]          # [D, 256]
    wq = np.ascontiguousarray(
        wqs.reshape(MC, 128, 256).transpose(1, 0, 2).reshape(128, MC * 256)
    ).astype(BF)
    # kt per head: only the 64 real rows; junk rows are memset on-chip
    kt = np.ascontiguousarray(K[b, hs].transpose(0, 2, 1)).astype(BF)
    # vext: [HPC, 128 kv, kt-tile, 65] where cols 0..63 = centered V,
    # col 64 = 1.0 (softmax denominator)
    vh = V[b, hs].astype(np.float32)                      # [HPC, NKV, DK]
    mv = vh.mean(axis=1, dtype=np.float32)                # [HPC, DK]
    vc = (vh - mv[:, None, :]).reshape(HPC, KT_TILES, 128, DK).transpose(0, 2, 1, 3)
    vext = np.ones((HPC, 128, KT_TILES, VW), np.float32)
    vext[:, :, :, :DK] = vc
    vext = vext.reshape(HPC, 128, KT_TILES * VW).astype(BF)
    wos = Wo[HPC * DK * g:HPC * DK * (g + 1), :]          # [256, D]
    wo = np.ascontiguousarray(
        wos.reshape(2, 128, D).transpose(1, 0, 2).reshape(128, 2 * D)
    ).astype(BF)
    kvm = kv_pad_mask[b, 0, 0].astype(bool)
    ngcol = np.where(kvm, 0.0, -1e9).astype(np.float32).reshape(KT_TILES, 128).T
    negm = np.ascontiguousarray(ngcol)
    # Schraudolph variant of the mask bias: scalar2 = A16*(bias) + B16
    negs = np.ascontiguousarray(
        (A16 * ngcol + np.float32(B16)).astype(np.float32))
    qmr = np.ascontiguousarray(
        q_pad_mask[b, 0, :, 0].astype(np.float32).reshape(1, NQ)).astype(BF)
    return dict(xt=xt, wq=wq, kt=kt, vext=vext, wo=wo, negm=negm, negs=negs,
                qmr=qmr)


def _install_ntff_hook():
    """The axon NTFF profile hook normally lives in antenv.axon_hooks,
    which this image lacks. Recreate it from trn_agent_boot so
    trace=True profiling works."""
    import sys
    import types
    try:
        from antenv.axon_hooks import get_axon_ntff_profile_hook  # noqa: F401
        return
    except ImportError:
        pass
    try:
        from trn_agent_boot.trn_boot import _ntff_profile_via_ctypes
        hook = _ntff_profile_via_ctypes("/opt/axon/libaxon_pjrt.so")
    except Exception:
        hook = None
    m = types.ModuleType("antenv.axon_hooks")
    m.get_axon_ntff_profile_hook = lambda: hook
    m.set_axon_ntff_profile_hook = lambda h: None
    sys.modules["antenv.axon_hooks"] = m


def kernel(x, K, V, Wq, Wo, kv_pad_mask, q_pad_mask, _trace=False):
    if _trace:
        _install_ntff_hook()
    nc = _get_program()
    x = np.asarray(x)
    K = np.asarray(K)
    V = np.asarray(V)
    Wq = np.asarray(Wq)
    Wo = np.asarray(Wo)
    kv_pad_mask = np.asarray(kv_pad_mask)
    q_pad_mask = np.asarray(q_pad_mask)
    in_maps = [_core_inputs(c, x, K, V, Wq, Wo, kv_pad_mask, q_pad_mask)
               for c in range(B * CPB)]
    res = run_bass_kernel_spmd(nc, in_maps, list(range(B * CPB)), trace=_trace)
    kernel._last_exec_ns = res.exec_time_ns
    kernel._last_results = res
    out = np.empty((B, NQ, D), np.float32)
    for b in range(B):
        acc = res.results[b * CPB]["out"].astype(np.float32)
        for j in range(1, CPB):
            acc = acc + res.results[b * CPB + j]["out"].astype(np.float32)
        # host-side add of the centered-V mean term: meanV @ Wo is a
        # constant row (covers both valid rows' mean part and masked-q
        # rows' uniform-softmax output)
        mv_all = V[b].astype(np.float32).mean(axis=1).reshape(1, D)
        acc = acc + mv_all @ Wo.astype(np.float32)
        out[b] = acc
    return out


kernel._last_exec_ns = None
kernel._last_results = None


# revision 29
# speedup vs baseline: 1.1183x; 1.1183x over previous
"""Cross multi-head attention on 8 Trainium2 NeuronCores.

Sharding: batch x head-group. Core c handles batch b = c//4 and heads
4*(c%4) .. 4*(c%4)+3. Wq is tensor-sharded by head (columns), Wo by its
input (head) dim (rows); the 4 partial outputs per batch are summed on
the host.

v4 design notes (205us v3 -> this):
- Phase A packs 2 heads per stationary (no DKP column padding on Wq):
  QT pair tiles are [128 = 2 heads x 64, NQ].  The scores matmul still
  contracts over 128 partitions: rows of the OTHER head in the pair act
  as junk rows, and the kt stationary carries eps rows on the opposite
  half (memset on-chip, not DMAed), so the junk contributes ~1e-4 to
  scaled scores while the PE array reads as fully busy (clock-gate).
- vext carries 65 columns per kv tile (64 centered-V + 1 ones for the
  softmax denominator) instead of 128 eps-padded ones: PV output is
  [65, q] and the denominator sits in PSUM row 64.
- Phase B runs the PV matmuls TWO kt tiles behind the scores matmuls
  and the Schraudolph/DVE exp tiles sit EARLY in the block while the
  last two ACT exp tiles are split in half, so the next block's first
  scores matmul never waits on a PSUM bank held by a late exp.
- exp split: ACT exact Exp 10 tiles, DVE int16-Schraudolph 6 tiles
  (measured end-to-end rel err 1.7e-2 < 2e-2).
- Norm per block (deferred into the next block): ACT Ln[1,QB] ->
  ACT Exp(-ln)[1,QB] -> GPS raq = ra*qmask [1,QB] -> GPS
  partition_broadcast [64,QB] -> DVE scalar_tensor_tensor
  dst = attn(PSUM) * raq64(SBUF).
- DMA: batched instructions (one per logical tensor where possible),
  kt ships only the 64 real rows, output is fp16.  Host sums the 4
  partials per batch in f32 and adds the centered-V mean term
  meanV @ Wo.
"""

import numpy as np
import ml_dtypes

import concourse.bass as bass
import concourse.mybir as mybir
import concourse.tile as tile
from concourse.bass_utils import run_bass_kernel_spmd

F32 = mybir.dt.float32
F16 = mybir.dt.float16
BF16 = mybir.dt.bfloat16
I16 = mybir.dt.int16
AF = mybir.ActivationFunctionType
ALU = mybir.AluOpType
BF = ml_dtypes.bfloat16

B, H, NQ, NKV, D, DK = 2, 16, 2048, 2048, 1024, 64
EPS = 1e-4           # junk-row constant in kt: junk q rows (the pair head's
                     # real data, ~N(0,1)) hit these eps rows -> ~1e-5 noise
HPC = 4              # heads per core
CPB = 4              # cores per batch
KT_TILES = NKV // 128
QT_TILES = NQ // 128
MC = D // 128        # model-dim chunks
QB = 1024            # q block width for scores/exp/attn
NQB = NQ // QB
SCALE = 0.125        # 1/sqrt(DK)
VW = 65              # vext columns per kv tile (64 centered V + 1 ones)

# Schraudolph fast-exp (int16 / bf16 bitpattern): exp(x) ~ bitcast_bf16(
#   int16(A16*x + B16)).  A16 = 2^7/ln2; B16 tuned for min max-rel-err.
A16 = 184.66500888183135
B16 = 127.0 * 128.0 - 4.5
# kt tiles handled by Schraudolph fast-exp on the DVE; early positions so
# the block tail is all-ACT (GpSimd cannot read PSUM -> no exp there)
DVE_EXP_KT = frozenset((1, 3, 5, 7, 9, 11))
SPLIT_EXP_KT = frozenset((14, 15))  # ACT tiles split in half for early
                                    # PSUM release at the block boundary


def _split_excess_waits(nc, limit=1):
    """This walrus build rejects instructions carrying several sem waits.
    Move excess waits onto standalone EventSemaphore instructions placed
    directly before the offender on the same (FIFO) engine queue."""
    n = 0
    for f in nc.m.functions:
        for bb in f.blocks:
            out = []
            for inst in bb.instructions:
                si = inst.sync_info
                waits = list(si.on_wait) if si is not None else []
                if len(waits) > limit:
                    excess, keep = waits[:-limit], waits[-limit:]
                    for w in excess:
                        n += 1
                        out.append(mybir.InstEventSemaphore(
                            name=f"wsplit-{n}-{inst.name}",
                            engine=inst.engine,
                            ins=[], outs=[],
                            sync_info=mybir.SyncInfo(on_wait=[w], on_update=[]),
                        ))
                    si.on_wait = keep
                out.append(inst)
            bb.instructions = out
    return n


def _build_program():
    nc = bass.Bass("TRN2", target_bir_lowering=False, debug=False, num_devices=8)

    d_xt = nc.declare_dram_parameter("xt", [D, NQ], BF16, isOutput=False)
    d_wq = nc.declare_dram_parameter("wq", [128, MC * 256], BF16, isOutput=False)
    d_kt = nc.declare_dram_parameter("kt", [HPC, DK, NKV], BF16, isOutput=False)
    d_vx = nc.declare_dram_parameter("vext", [HPC, 128, KT_TILES * VW], BF16, isOutput=False)
    d_wo = nc.declare_dram_parameter("wo", [128, 2 * D], BF16, isOutput=False)
    d_ng = nc.declare_dram_parameter("negm", [128, KT_TILES], F32, isOutput=False)
    d_ngs = nc.declare_dram_parameter("negs", [128, KT_TILES], F32, isOutput=False)
    d_qm = nc.declare_dram_parameter("qmb", [DK, NQ], BF16, isOutput=False)
    d_out = nc.declare_dram_parameter("out", [NQ, D], F16, isOutput=True)

    with tile.TileContext(nc) as tc:
        with (
            tc.tile_pool(name="persist", bufs=1) as pp,
            tc.tile_pool(name="fin", bufs=1) as fp,
            tc.tile_pool(name="sb_small", bufs=2) as sp,
        ):
            # ---- persistent tiles ----
            t_kt = [pp.tile([128, NKV], BF16, name=f"kt{h}", tag=f"kt{h}")
                    for h in range(HPC)]
            # memset the junk halves of kt on-chip (saves 1MB of DMA);
            # even heads of a pair use rows 0..63, odd heads rows 64..127
            for h in range(HPC):
                if h % 2 == 0:
                    nc.gpsimd.memset(t_kt[h][DK:128, :], EPS)
                else:
                    nc.gpsimd.memset(t_kt[h][0:DK, :], EPS)

            # ---- input loads (ordered so Phase A can start immediately:
            # wq chunk 0 and xt chunk 0 first, then kt/vx interleaved with
            # the later xt chunks so Phase B's first head is ready early) ----
            t_xt = [pp.tile([128, NQ], BF16, name=f"xt{mc}", tag=f"xt{mc}")
                    for mc in range(MC)]
            t_wq = pp.tile([128, MC * 256], BF16, name="wq", tag="wq")
            t_vx = [pp.tile([128, KT_TILES * VW], BF16, name=f"vx{h}", tag=f"vx{h}")
                    for h in range(HPC)]
            nc.sync.dma_start(out=t_wq[:, 0:256], in_=d_wq[:, 0:256])
            nc.sync.dma_start(out=t_xt[0][:, :], in_=d_xt[0:128, :])
            nc.sync.dma_start(out=t_wq[:, 256:], in_=d_wq[:, 256:])
            t_ng = pp.tile([128, KT_TILES], F32, name="negm", tag="negm")
            nc.sync.dma_start(out=t_ng[:, :], in_=d_ng[:, :])
            t_ngs = pp.tile([128, KT_TILES], F32, name="negs", tag="negs")
            nc.sync.dma_start(out=t_ngs[:, :], in_=d_ngs[:, :])
            t_qm = pp.tile([DK, NQ], BF16, name="qmb", tag="qmb")
            nc.sync.dma_start(out=t_qm[:, :], in_=d_qm[:, :])

            def load_kt_vx(h):
                if h % 2 == 0:
                    nc.sync.dma_start(out=t_kt[h][0:DK, :], in_=d_kt[h, :, :])
                else:
                    nc.sync.dma_start(out=t_kt[h][DK:128, :], in_=d_kt[h, :, :])
                nc.sync.dma_start(out=t_vx[h][:, :], in_=d_vx[h, :, :])

            for mc in range(1, MC):
                nc.sync.dma_start(out=t_xt[mc][:, :],
                                  in_=d_xt[mc * 128:(mc + 1) * 128, :])
                if mc == 5:
                    load_kt_vx(0)
                elif mc == 7:
                    load_kt_vx(1)
            load_kt_vx(2)
            load_kt_vx(3)
            t_wo = pp.tile([128, 2 * D], BF16, name="wo", tag="wo")
            nc.sync.dma_start(out=t_wo[:, :], in_=d_wo[:, :])

            t_qt = [pp.tile([128, NQ], BF16, name=f"qt{p}", tag=f"qt{p}")
                    for p in range(2)]
            t_on = pp.tile([1, DK], BF16, name="ones", tag="ones")
            nc.vector.memset(t_on[:, :], 1.0)
            t_fAB = fp.tile([128, NQ], BF16, name="attnAB", tag="attnAB")
            t_fCD = fp.tile([128, NQ], BF16, name="attnCD", tag="attnCD")

            # ---- Phase A: QT_pair = Wq_pair^T @ x^T (mc-outer, DMA-paced) ----
            with tc.tile_pool(name="ps_q", bufs=1, space="PSUM") as pqp:
                ps_q = [pqp.tile([128, NQ], F32, name=f"ps_q{p}", tag=f"ps_q{p}")
                        for p in range(2)]
                for mc in range(MC):
                    for pr in range(2):
                        for nb in range(NQ // 512):
                            nc.tensor.matmul(
                                ps_q[pr][:, nb * 512:(nb + 1) * 512],
                                t_wq[:, mc * 256 + pr * 128:mc * 256 + (pr + 1) * 128],
                                t_xt[mc][:, nb * 512:(nb + 1) * 512],
                                start=(mc == 0), stop=(mc == MC - 1),
                            )
                # PSUM -> SBUF bf16, split across engines (A->B critical
                # path; GpSimd cannot read PSUM)
                nc.vector.tensor_copy(t_qt[0][:, 0:NQ // 2], ps_q[0][:, 0:NQ // 2])
                nc.scalar.copy(t_qt[0][:, NQ // 2:], ps_q[0][:, NQ // 2:])
                nc.vector.tensor_copy(t_qt[1][:, 0:NQ // 2], ps_q[1][:, 0:NQ // 2])
                nc.scalar.copy(t_qt[1][:, NQ // 2:], ps_q[1][:, NQ // 2:])

            # ---- Phase B: per (q-block, head) attention ----
            # PV matmuls run TWO kt tiles behind scores; normalization of
            # block i is deferred into block i+1 so no engine queue stalls.
            # Pools are managed manually so Phase C can open in the scores
            # pool's banks while the last block's norm is still draining.
            prp = tc.alloc_tile_pool(name="probs", bufs=2)
            scp = tc.alloc_tile_pool(name="ps_sc", bufs=2, space="PSUM",
                                     side="right")
            atp = tc.alloc_tile_pool(name="ps_at", bufs=2, space="PSUM")
            if True:
                pend = []  # deferred norm steps of the previous block
                for qh in range(NQB):
                    q0 = qh * QB
                    for h in range(HPC):
                        dst = t_fAB if h < 2 else t_fCD
                        rbase = (h % 2) * DK
                        pr = h // 2
                        probsT = prp.tile([128, KT_TILES * QB], BF16,
                                          name="probsT", tag="probsT")
                        # rows 0..63 attn accum, row 64 denominator, rows
                        # 64..127 later overwritten with the ra broadcast
                        at_ps = atp.tile([128, QB], F32, name="at_ps", tag="at_ps")

                        def at_mms(kt, at_ps=at_ps, probsT=probsT, h=h):
                            for nb in range(QB // 512):
                                nc.tensor.matmul(
                                    at_ps[0:VW, nb * 512:(nb + 1) * 512],
                                    t_vx[h][:, kt * VW:(kt + 1) * VW],
                                    probsT[:, kt * QB + nb * 512:kt * QB + (nb + 1) * 512],
                                    start=(kt == 0), stop=(kt == KT_TILES - 1),
                                )

                        for kt in range(KT_TILES):
                            sc = scp.tile([128, QB], F32, name="sc", tag="sc")
                            split = kt in SPLIT_EXP_KT
                            for nb in range(QB // 512):
                                nc.tensor.matmul(
                                    sc[:, nb * 512:(nb + 1) * 512],
                                    t_kt[h][:, kt * 128:(kt + 1) * 128],
                                    t_qt[pr][:, q0 + nb * 512:q0 + (nb + 1) * 512],
                                    start=True, stop=True,
                                )
                                if split:
                                    nc.scalar.activation(
                                        probsT[:, kt * QB + nb * 512:kt * QB + (nb + 1) * 512],
                                        sc[:, nb * 512:(nb + 1) * 512], AF.Exp,
                                        bias=t_ng[:, kt:kt + 1], scale=SCALE)
                            if not split:
                                pslice = probsT[:, kt * QB:(kt + 1) * QB]
                                if kt in DVE_EXP_KT:
                                    nc.vector.tensor_scalar(
                                        pslice.bitcast(I16), sc[:, :],
                                        A16 * SCALE, t_ngs[:, kt:kt + 1],
                                        ALU.mult, ALU.add,
                                    )
                                else:
                                    nc.scalar.activation(pslice, sc[:, :], AF.Exp,
                                                         bias=t_ng[:, kt:kt + 1],
                                                         scale=SCALE)
                            # drain one deferred norm step of the previous
                            # block every couple of kt tiles
                            if kt in (1, 3, 5, 9, 11) and pend:
                                pend.pop(0)()
                            if kt >= 2:
                                at_mms(kt - 2)
                        at_mms(KT_TILES - 2)
                        at_mms(KT_TILES - 1)

                        # deferred normalization steps (run inside next block):
                        #   tmp = attn * qmask, PSUM->SBUF  [64,QB] bf16 (DVE)
                        #   ln  = Ln(denom row)             [1,QB] f32   (ACT)
                        #   ra  = Exp(-ln)                  [1,QB] bf16  (ACT)
                        #   bcast ra -> at_ps[64:128]       (PE ones matmul)
                        #   dst = tmp * at_ps[64:128]       (DVE, one PSUM op)
                        def mk_norm(at_ps=at_ps, dst=dst, rbase=rbase, q0=q0):
                            st = {}

                            def s_tmp():
                                t_tmp = sp.tile([DK, QB], BF16, name="tmp",
                                                tag="tmp")
                                nc.vector.tensor_mul(t_tmp[:, :],
                                                     at_ps[0:DK, :],
                                                     t_qm[:, q0:q0 + QB])
                                st["tmp"] = t_tmp

                            def s_ln():
                                t_ln = sp.tile([1, QB], F32, name="ln", tag="ln")
                                nc.scalar.activation(t_ln[:, :],
                                                     at_ps[DK:DK + 1, :], AF.Ln)
                                st["ln"] = t_ln

                            def s_ra():
                                t_ra = sp.tile([1, QB], BF16, name="ra", tag="ra")
                                nc.scalar.activation(t_ra[:, :], st["ln"][:, :],
                                                     AF.Exp, scale=-1.0)
                                st["ra"] = t_ra

                            def s_bc():
                                for nb in range(QB // 512):
                                    s = slice(nb * 512, (nb + 1) * 512)
                                    nc.tensor.matmul(at_ps[DK:2 * DK, s],
                                                     t_on[:, :],
                                                     st["ra"][:, s],
                                                     start=True, stop=True)

                            def s_fin():
                                nc.vector.tensor_mul(
                                    dst[rbase:rbase + DK, q0:q0 + QB],
                                    st["tmp"][:, :], at_ps[DK:2 * DK, :])

                            return [s_tmp, s_ln, s_ra, s_bc, s_fin]

                        pend = mk_norm()

            # ---- Phase C: out = attnT_final^T @ Wo_slice ----
            # The scores pool is released and Phase C opens in its banks;
            # the last block's deferred norm drains while C's first tiles
            # (which don't depend on it) run, and its PE broadcast matmul
            # is emitted after C's first matmuls so the PE FIFO never
            # stalls on the ACT Ln/Exp chain.
            scp.release()
            pop = tc.alloc_tile_pool(name="ps_out", bufs=2, space="PSUM")
            op = tc.alloc_tile_pool(name="outsb", bufs=3)
            for s in pend[:3]:  # s_tmp, s_ln, s_ra (DVE/ACT only)
                s()
            pend = pend[3:]
            for qt_i in range(QT_TILES):
                if qt_i == 2:
                    for s in pend:  # s_bc, s_fin
                        s()
                    pend = []
                qs = slice(qt_i * 128, (qt_i + 1) * 128)
                po = pop.tile([128, D], F32, name="po", tag="po")
                for nb in range(D // 512):
                    s = slice(nb * 512, (nb + 1) * 512)
                    nc.tensor.matmul(po[:, s], t_fAB[:, qs], t_wo[:, s],
                                     start=True, stop=False)
                    nc.tensor.matmul(po[:, s], t_fCD[:, qs], t_wo[:, D + s.start:D + s.stop],
                                     start=False, stop=True)
                t_out = op.tile([128, D], F16, name="t_out", tag="t_out")
                if qt_i % 2 == 0 or qt_i >= QT_TILES - 2:
                    nc.vector.tensor_copy(t_out[:, :], po[:, :])
                else:
                    nc.scalar.copy(t_out[:, :], po[:, :])
                nc.sync.dma_start(out=d_out[qs, :], in_=t_out[:, :])
            pop.release()
            atp.release()
            op.release()
            prp.release()

    _split_excess_waits(nc, limit=1)
    return nc


_PROGRAM = None


def _get_program():
    global _PROGRAM
    if _PROGRAM is None:
        _PROGRAM = _build_program()
    return _PROGRAM


def _core_inputs(c, x, K, V, Wq, Wo, kv_pad_mask, q_pad_mask):
    b = c // CPB
    g = c % CPB
    hs = slice(HPC * g, HPC * g + HPC)
    xt = np.ascontiguousarray(x[b].T).astype(BF)
    # Wq columns for this group's 4 heads, merged mc-major: [128, MC*256]
    wqs = Wq[:, HPC * DK * g:HPC * DK * (g + 1)]          # [D, 256]
    wq = np.ascontiguousarray(
        wqs.reshape(MC, 128, 256).transpose(1, 0, 2).reshape(128, MC * 256)
    ).astype(BF)
    # kt per head: only the 64 real rows; junk rows are memset on-chip
    kt = np.ascontiguousarray(K[b, hs].transpose(0, 2, 1)).astype(BF)
    # vext: [HPC, 128 kv, kt-tile, 65] where cols 0..63 = centered V,
    # col 64 = 1.0 (softmax denominator)
    vh = V[b, hs].astype(np.float32)                      # [HPC, NKV, DK]
    mv = vh.mean(axis=1, dtype=np.float32)                # [HPC, DK]
    vc = (vh - mv[:, None, :]).reshape(HPC, KT_TILES, 128, DK).transpose(0, 2, 1, 3)
    vext = np.ones((HPC, 128, KT_TILES, VW), np.float32)
    vext[:, :, :, :DK] = vc
    vext = vext.reshape(HPC, 128, KT_TILES * VW).astype(BF)
    wos = Wo[HPC * DK * g:HPC * DK * (g + 1), :]          # [256, D]
    wo = np.ascontiguousarray(
        wos.reshape(2, 128, D).transpose(1, 0, 2).reshape(128, 2 * D)
    ).astype(BF)
    kvm = kv_pad_mask[b, 0, 0].astype(bool)
    ngcol = np.where(kvm, 0.0, -1e9).astype(np.float32).reshape(KT_TILES, 128).T
    negm = np.ascontiguousarray(ngcol)
    # Schraudolph variant of the mask bias: scalar2 = A16*(bias) + B16
    negs = np.ascontiguousarray(
        (A16 * ngcol + np.float32(B16)).astype(np.float32))
    qm = q_pad_mask[b, 0, :, 0].astype(np.float32).reshape(1, NQ)
    qmb = np.ascontiguousarray(np.broadcast_to(qm, (DK, NQ))).astype(BF)
    return dict(xt=xt, wq=wq, kt=kt, vext=vext, wo=wo, negm=negm, negs=negs,
                qmb=qmb)


def _install_ntff_hook():
    """The axon NTFF profile hook normally lives in antenv.axon_hooks,
    which this image lacks. Recreate it from trn_agent_boot so
    trace=True profiling works."""
    import sys
    import types
    try:
        from antenv.axon_hooks import get_axon_ntff_profile_hook  # noqa: F401
        return
    except ImportError:
        pass
    try:
        from trn_agent_boot.trn_boot import _ntff_profile_via_ctypes
        hook = _ntff_profile_via_ctypes("/opt/axon/libaxon_pjrt.so")
    except Exception:
        hook = None
    m = types.ModuleType("antenv.axon_hooks")
    m.get_axon_ntff_profile_hook = lambda: hook
    m.set_axon_ntff_profile_hook = lambda h: None
    sys.modules["antenv.axon_hooks"] = m


def kernel(x, K, V, Wq, Wo, kv_pad_mask, q_pad_mask, _trace=False):
    if _trace:
        _install_ntff_hook()
    nc = _get_program()
    x = np.asarray(x)
    K = np.asarray(K)
    V = np.asarray(V)
    Wq = np.asarray(Wq)
    Wo = np.asarray(Wo)
    kv_pad_mask = np.asarray(kv_pad_mask)
    q_pad_mask = np.asarray(q_pad_mask)
    in_maps = [_core_inputs(c, x, K, V, Wq, Wo, kv_pad_mask, q_pad_mask)
               for c in range(B * CPB)]
    res = run_bass_kernel_spmd(nc, in_maps, list(range(B * CPB)), trace=_trace)
    kernel._last_exec_ns = res.exec_time_ns
    kernel._last_results = res
    out = np.empty((B, NQ, D), np.float32)
    for b in range(B):
        acc = res.results[b * CPB]["out"].astype(np.float32)
        for j in range(1, CPB):
            acc = acc + res.results[b * CPB + j]["out"].astype(np.float32)
        # host-side add of the centered-V mean term: meanV @ Wo is a
        # constant row (covers both valid rows' mean part and masked-q
        # rows' uniform-softmax output)
        mv_all = V[b].astype(np.float32).mean(axis=1).reshape(1, D)
        acc = acc + mv_all @ Wo.astype(np.float32)
        out[b] = acc
    return out


kernel._last_exec_ns = None
kernel._last_results = None
